# revision 1
# baseline (speedup 1.0000x reference)
"""Trainium2 Bass kernel for hetero-GNN (2x ResGatedGraphConv + segment-mean pooling + MLP).

Sharding: destination-node range per core; each core processes the edges whose
dst falls in its range (edge lists sorted/bucketed by dst on host — index
marshalling only). All model arithmetic runs on device:
  - per-edge fused matmul: [x_src.T ; ea ; 1 ; x_dst.T] @ W_aug
      -> [q+k+2e+bias | v+e+bias]  (one constant-weight matmul)
  - sigmoid (ACT), gated message (DVE)
  - scatter-add via one-hot matmul accumulated in per-bucket PSUM
  - skip connection + relu, segment-mean pooling via one-hot matmul
  - cross-core AllReduce of pooled partials, 4-layer MLP head.
"""
import sys
import types
import numpy as np

NCORES = 8
G = 128
H = 64
F = 16
NC_N = 100000
NB_N = 200000
BUCKET = 128
GRP = 4  # 128-edge sub-tiles per matmul group
LAST_EXEC_NS = None


def _install_ntff_shim():
    if 'antenv.axon_hooks' in sys.modules:
        return
    try:
        mod = types.ModuleType('antenv.axon_hooks')
        _h = [None]
        mod.set_axon_ntff_profile_hook = lambda h: _h.__setitem__(0, h)
        mod.get_axon_ntff_profile_hook = lambda: _h[0]
        sys.modules['antenv.axon_hooks'] = mod
        import antenv
        antenv.axon_hooks = mod
        from trn_agent_boot.trn_boot import _ntff_profile_via_ctypes
        mod.set_axon_ntff_profile_hook(
            _ntff_profile_via_ctypes('/opt/axon/libaxon_pjrt.so'))
    except Exception:
        pass


def _prep_relation(x_src, x_dst, src, dst, ea, D):
    """Host marshalling for one relation: per-core padded edge streams."""
    nbuck = (D + BUCKET - 1) // BUCKET
    order = np.argsort(dst, kind="stable")
    src_s, dst_s, ea_s = src[order], dst[order], ea[order, 0]
    core_of = dst_s // D
    buck_of = (dst_s % D) // BUCKET
    counts = np.zeros((NCORES, nbuck), np.int64)
    np.add.at(counts, (core_of, buck_of), 1)
    subtiles = np.maximum((counts.max(axis=0) + 127) // 128, 1)   # [nbuck]
    ntot = int(subtiles.sum()) * 128
    starts = np.zeros(nbuck + 1, np.int64)
    starts[1:] = np.cumsum(subtiles) * 128
    xs = x_src.astype(np.float16)
    xd = x_dst.astype(np.float16)
    per_core = []
    core_bounds = np.searchsorted(core_of, np.arange(NCORES + 1))
    for m in range(NCORES):
        lo, hi = core_bounds[m], core_bounds[m + 1]
        c_src, c_dst, c_ea = src_s[lo:hi], dst_s[lo:hi], ea_s[lo:hi]
        c_buck = (c_dst % D) // BUCKET
        pos_in_bucket = np.arange(len(c_src)) - np.searchsorted(c_buck, c_buck)
        slot = starts[c_buck] + pos_in_bucket
        xt = np.zeros((35, ntot), np.float16)
        ld = np.full(ntot, -1.0, np.float32)
        xt[0:16, slot] = xs[c_src].T
        xt[16, slot] = c_ea.astype(np.float16)
        xt[17, slot] = 1.0
        xt[18:34, slot] = xd[c_dst].T
        ld[slot] = (c_dst % D) % BUCKET
        per_core.append({
            "xt": xt,
            "ld": np.ascontiguousarray(ld.reshape(-1, 128).T),  # [128, nsub]
        })
    return {"nbuck": nbuck, "subtiles": subtiles, "ntot": ntot,
            "per_core": per_core}


def kernel(**inputs):
    _install_ntff_shim()
    import concourse.bass as bass  # noqa: F401
    import concourse.bacc as bacc
    import concourse.mybir as mybir
    import concourse.tile as tile
    from concourse.bass_utils import run_bass_kernel_spmd

    F32 = mybir.dt.float32
    F16 = mybir.dt.float16
    AF = mybir.ActivationFunctionType
    OP = mybir.AluOpType

    ii = {k: np.asarray(v) for k, v in inputs.items()}
    Dc, Db = NC_N // NCORES, NB_N // NCORES

    rel_c = _prep_relation(ii["x_x"], ii["x_c"], ii["src_ac"].astype(np.int64),
                           ii["dst_ac"].astype(np.int64), ii["ea_ac"], Dc)
    rel_b = _prep_relation(ii["x_c"], ii["x_b"], ii["src_cb"].astype(np.int64),
                           ii["dst_cb"].astype(np.int64), ii["ea_cb"], Db)

    def phase_a(x_dst, D, m):
        sl = x_dst[m * D:(m + 1) * D]
        a = np.zeros((17, D), np.float16)
        a[0:16] = sl.T.astype(np.float16)
        a[16] = 1.0
        return a

    def batch_layout(batch, D, m):
        nbuck = (D + BUCKET - 1) // BUCKET
        sl = batch[m * D:(m + 1) * D].astype(np.float32)
        padded = np.full(nbuck * BUCKET, -1.0, np.float32)
        padded[:D] = sl
        return np.ascontiguousarray(padded.reshape(nbuck, BUCKET).T)  # [128, nbuck]

    cnt_c = np.bincount(ii["batch_c"].astype(np.int64), minlength=G).astype(np.float32)
    cnt_b = np.bincount(ii["batch_b"].astype(np.int64), minlength=G).astype(np.float32)
    recip = np.stack([1.0 / np.maximum(cnt_c, 1.0),
                      1.0 / np.maximum(cnt_b, 1.0)]).astype(np.float16)  # [2, G]

    def waug(rel):
        Wq, Wv, Wk = ii[f"Wq_{rel}"], ii[f"Wv_{rel}"], ii[f"Wk_{rel}"]
        We = ii[f"We_{rel}"][0]
        bq, bv, bk, be = (ii[f"bq_{rel}"], ii[f"bv_{rel}"],
                          ii[f"bk_{rel}"], ii[f"be_{rel}"])
        w = np.zeros((35, 128), np.float32)
        w[0:16, 0:64] = Wq; w[0:16, 64:128] = Wv
        w[16, 0:64] = 2 * We; w[16, 64:128] = We
        w[17, 0:64] = bq + bk + 2 * be; w[17, 64:128] = bv + be
        w[18:34, 0:64] = Wk
        return w.astype(np.float16)

    def wskip(rel):
        w = np.zeros((17, 64), np.float32)
        w[0:16] = ii[f"Wskip_{rel}"]
        w[16] = ii[f"bconv_{rel}"]
        return w.astype(np.float16)

    iota_row = np.tile(np.arange(BUCKET, dtype=np.float32), (128, 1))
    iota_g = np.tile(np.arange(G, dtype=np.float32), (128, 1))
    mlp_w = {
        "W1": ii["W1"].astype(np.float16), "W2": ii["W2"].astype(np.float16),
        "W3": ii["W3"].astype(np.float16), "Wout": ii["Wout"].astype(np.float16),
        "b1": ii["b1"].astype(np.float32).reshape(64, 1),
        "b2": ii["b2"].astype(np.float32).reshape(64, 1),
        "b3": ii["b3"].astype(np.float32).reshape(64, 1),
        "bout": ii["bout"].astype(np.float32).reshape(1, 1),
    }

    # ---------------- device program ----------------
    nc = bacc.Bacc("TRN2", target_bir_lowering=False, debug=False,
                   num_devices=NCORES)

    def din(name, arr0):
        return nc.dram_tensor(name, list(arr0.shape),
                              mybir.dt.from_np(arr0.dtype), kind="ExternalInput")

    h = {}
    h["xt_c"] = din("xt_c", rel_c["per_core"][0]["xt"])
    h["xt_b"] = din("xt_b", rel_b["per_core"][0]["xt"])
    h["ld_c"] = din("ld_c", rel_c["per_core"][0]["ld"])
    h["ld_b"] = din("ld_b", rel_b["per_core"][0]["ld"])
    h["pa_c"] = din("pa_c", phase_a(ii["x_c"], Dc, 0))
    h["pa_b"] = din("pa_b", phase_a(ii["x_b"], Db, 0))
    h["bt_c"] = din("bt_c", batch_layout(ii["batch_c"], Dc, 0))
    h["bt_b"] = din("bt_b", batch_layout(ii["batch_b"], Db, 0))
    h["waug_c"] = din("waug_c", waug("ac"))
    h["waug_b"] = din("waug_b", waug("cb"))
    h["wskip_c"] = din("wskip_c", wskip("ac"))
    h["wskip_b"] = din("wskip_b", wskip("cb"))
    h["iota"] = din("iota", iota_row)
    h["iotag"] = din("iotag", iota_g)
    h["recip"] = din("recip", recip)
    sel2 = np.zeros((2, 128), np.float16); sel2[0, 0:64] = 1; sel2[1, 64:128] = 1
    h["ones2"] = din("ones2", sel2)
    for k, v in mlp_w.items():
        h["mlp_" + k] = din("mlp_" + k, v)
    out_h = nc.dram_tensor("out", [1, G], F32, kind="ExternalOutput")

    with tile.TileContext(nc) as tc:
        with tc.tile_pool(name="const", bufs=1) as cp, \
             tc.tile_pool(name="acc", bufs=1) as accp, \
             tc.tile_pool(name="stream", bufs=3) as sp, \
             tc.tile_pool(name="work", bufs=3) as wp, \
             tc.tile_pool(name="psum", bufs=2, space="PSUM") as pp, \
             tc.tile_pool(name="psA", bufs=1, space="PSUM") as ppA, \
             tc.tile_pool(name="dram", bufs=1, space="DRAM") as dp:

            iota_t = cp.tile([128, BUCKET], F32, tag="iota_t")
            nc.sync.dma_start(iota_t[:], h["iota"].ap())
            iota4_t = cp.tile([128, GRP, BUCKET], F32, tag="iota4_t")
            for _j in range(GRP):
                nc.vector.tensor_copy(iota4_t[:, _j, :], iota_t[:])
            iotag_t = cp.tile([128, G], F32, tag="iotag_t")
            nc.sync.dma_start(iotag_t[:], h["iotag"].ap())

            pooled_ps = ppA.tile([128, G], F32, tag="pooled_ps")

            def relation(tag, rel, D, row_off):
                nbuck = rel["nbuck"]
                subtiles = rel["subtiles"]
                w_t = cp.tile([35, 128], F16, name=f"waug_{tag}", tag=f"waug_{tag}")
                nc.sync.dma_start(w_t[:], h[f"waug_{tag}"].ap())
                ws_t = cp.tile([17, 64], F16, name=f"wskip_{tag}", tag=f"wskip_{tag}")
                nc.sync.dma_start(ws_t[:], h[f"wskip_{tag}"].ap())

                agg = accp.tile([128, nbuck * 64], F32, name=f"agg_{tag}",
                                tag=f"agg_{tag}")
                pa_sb = accp.tile([17, D], F16, name=f"pa_{tag}", tag=f"pa_{tag}")
                nc.sync.dma_start(pa_sb[:], h[f"pa_{tag}"].ap())
                for b in range(nbuck):
                    w = min(BUCKET, D - b * BUCKET)
                    ps = pp.tile([128, 64], F32, name=f"skps_{tag}_{b}", tag="skps")
                    nc.tensor.matmul(ps[:w, :], pa_sb[:, b * BUCKET:b * BUCKET + w],
                                     ws_t[:], start=True, stop=True)
                    if w < BUCKET:
                        nc.vector.memset(agg[:, b * 64:(b + 1) * 64], 0.0)
                    nc.vector.tensor_copy(agg[:w, b * 64:(b + 1) * 64], ps[:w, :])

                xt_v = h[f"xt_{tag}"].ap()
                ld_v = h[f"ld_{tag}"].ap()
                sub0 = 0
                for b in range(nbuck):
                    nsub = int(subtiles[b])
                    bps = pp.tile([128, 64], F32, name=f"bps_{tag}_{b}", tag="bps")
                    s = 0
                    while s < nsub:
                        g = min(GRP, nsub - s)
                        e0 = (sub0 + s) * 128
                        xt_t = sp.tile([35, GRP * 128], F16, name=f"xt_{tag}_{b}_{s}",
                                       tag="xt")
                        nc.sync.dma_start(xt_t[:, :g * 128], xt_v[:, e0:e0 + g * 128])
                        ld_t = sp.tile([128, GRP], F32, name=f"ldt_{tag}_{b}_{s}",
                                       tag="ldt")
                        nc.sync.dma_start(ld_t[:, :g],
                                          ld_v[:, sub0 + s:sub0 + s + g])
                        sv = pp.tile([128, GRP * 128], F32, name=f"sv_{tag}_{b}_{s}",
                                     tag="sv")
                        for j in range(g):
                            nc.tensor.matmul(sv[:, j * 128:(j + 1) * 128],
                                             xt_t[:, j * 128:(j + 1) * 128],
                                             w_t[:], start=True, stop=True)
                        sv3 = sv[:].rearrange("p (a b) -> p a b", a=GRP)
                        gt = wp.tile([128, GRP, 64], F32, name=f"gt_{tag}_{b}_{s}",
                                     tag="gt")
                        nc.scalar.activation(gt[:, :g, :], sv3[:, :g, 0:64],
                                             AF.Sigmoid)
                        msg = wp.tile([128, GRP, 64], F16, name=f"msg_{tag}_{b}_{s}",
                                      tag="msg")
                        nc.vector.tensor_tensor(msg[:, :g, :], gt[:, :g, :],
                                                sv3[:, :g, 64:128], op=OP.mult)
                        oh4 = wp.tile([128, GRP, BUCKET], F16,
                                      name=f"oh_{tag}_{b}_{s}", tag="oh")
                        ld3 = ld_t[:, :g].rearrange("p (a o) -> p a o", o=1)
                        nc.vector.tensor_tensor(
                            oh4[:, :g, :], iota4_t[:, :g, :],
                            ld3.broadcast_to([128, g, BUCKET]),
                            op=OP.is_equal)
                        for j in range(g):
                            nc.tensor.matmul(bps[:], oh4[:, j, :], msg[:, j, :],
                                             start=(s + j == 0),
                                             stop=(s + j == nsub - 1),
                                             skip_group_check=True)
                        s += g
                    nc.vector.tensor_tensor(agg[:, b * 64:(b + 1) * 64],
                                            agg[:, b * 64:(b + 1) * 64], bps[:],
                                            op=OP.add)
                    sub0 += nsub

                h_sb = accp.tile([128, nbuck * 64], F16, name=f"h_{tag}",
                                 tag=f"h_{tag}")
                nc.scalar.activation(h_sb[:], agg[:], AF.Relu)
                bt_sb = accp.tile([128, nbuck], F32, name=f"bt_{tag}",
                                  tag=f"bt_{tag}")
                nc.sync.dma_start(bt_sb[:], h[f"bt_{tag}"].ap())
                for b in range(nbuck):
                    ohg = wp.tile([128, G], F16, name=f"ohg_{tag}_{b}", tag="ohg")
                    nc.vector.tensor_scalar(ohg[:], iotag_t[:], bt_sb[:, b:b + 1],
                                            None, OP.is_equal)
                    nc.tensor.matmul(pooled_ps[row_off:row_off + 64, :],
                                     h_sb[:, b * 64:(b + 1) * 64], ohg[:],
                                     start=(b == 0), stop=(b == nbuck - 1),
                                     skip_group_check=True)

            relation("c", rel_c, Dc, 0)
            relation("b", rel_b, Db, 64)

            pooled_sb = accp.tile([128, G], F32, tag="pooled_sb")
            nc.vector.tensor_copy(pooled_sb[:], pooled_ps[:])
            bounce_in = dp.tile([128, G], F32, tag="bounce_in")
            bounce_out = dp.tile([128, G], F32, tag="bounce_out")
            nc.sync.dma_start(bounce_in[:], pooled_sb[:])
            nc.gpsimd.collective_compute(
                "AllReduce", OP.add, replica_groups=[list(range(NCORES))],
                ins=[bounce_in.opt()], outs=[bounce_out.opt()])
            nc.sync.dma_start(pooled_sb[:], bounce_out[:])

            recip_sb = accp.tile([2, G], F16, tag="recip_sb")
            nc.sync.dma_start(recip_sb[:], h["recip"].ap())
            ones2_sb = accp.tile([2, 128], F16, tag="ones2_sb")
            nc.sync.dma_start(ones2_sb[:], h["ones2"].ap())
            rb_ps = ppA.tile([128, G], F32, tag="mlps")
            nc.tensor.matmul(rb_ps[:], ones2_sb[:], recip_sb[:],
                             start=True, stop=True)
            mean_sb = accp.tile([128, G], F16, tag="mean_sb")
            nc.vector.tensor_tensor(mean_sb[:], pooled_sb[:], rb_ps[:], op=OP.mult)

            mw, mb = {}, {}
            for k in ("W1", "W2", "W3", "Wout"):
                mw[k] = accp.tile(list(mlp_w[k].shape), F16, name=f"mw{k}",
                                  tag=f"mw{k}")
                nc.sync.dma_start(mw[k][:], h["mlp_" + k].ap())
            for k in ("b1", "b2", "b3", "bout"):
                mb[k] = accp.tile(list(mlp_w[k].shape), F32, name=f"mb{k}",
                                  tag=f"mb{k}")
                nc.sync.dma_start(mb[k][:], h["mlp_" + k].ap())

            hcur = mean_sb
            for li, (wk, bk) in enumerate((("W1", "b1"), ("W2", "b2"),
                                           ("W3", "b3"))):
                ps = ppA.tile([64, G], F32, name=f"mlp{li}", tag="mlps")
                nc.tensor.matmul(ps[:], mw[wk][:], hcur[:], start=True, stop=True)
                hn = accp.tile([64, G], F16, name=f"hn{li}", tag=f"hn{li}")
                nc.scalar.activation(hn[:], ps[:], AF.Relu, bias=mb[bk][:])
                hcur = hn
            ps_o = ppA.tile([1, G], F32, tag="mlps")
            nc.tensor.matmul(ps_o[:], mw["Wout"][:], hcur[:], start=True, stop=True)
            osb = accp.tile([1, G], F32, tag="osb")
            nc.scalar.activation(osb[:], ps_o[:], AF.Identity, bias=mb["bout"][:])
            nc.sync.dma_start(out_h.ap(), osb[:])

    nc.compile()

    in_maps = []
    for m in range(NCORES):
        in_maps.append({
            "xt_c": rel_c["per_core"][m]["xt"],
            "xt_b": rel_b["per_core"][m]["xt"],
            "ld_c": rel_c["per_core"][m]["ld"],
            "ld_b": rel_b["per_core"][m]["ld"],
            "pa_c": phase_a(ii["x_c"], Dc, m), "pa_b": phase_a(ii["x_b"], Db, m),
            "bt_c": batch_layout(ii["batch_c"], Dc, m),
            "bt_b": batch_layout(ii["batch_b"], Db, m),
            "waug_c": waug("ac"), "waug_b": waug("cb"),
            "wskip_c": wskip("ac"), "wskip_b": wskip("cb"),
            "iota": iota_row, "iotag": iota_g, "recip": recip,
            "ones2": sel2,
            **{"mlp_" + k: v for k, v in mlp_w.items()},
        })
    import os
    trace = bool(os.environ.get("KERNEL_TRACE"))
    res = run_bass_kernel_spmd(nc, in_maps, core_ids=list(range(NCORES)),
                               trace=trace)
    global LAST_EXEC_NS
    LAST_EXEC_NS = res.exec_time_ns
    return res.results[0]["out"].reshape(G).astype(np.float32)



# revision 8
# speedup vs baseline: 1.6759x; 1.6759x over previous
"""Trainium2 Bass kernel for hetero-GNN (2x ResGatedGraphConv + segment-mean pooling + MLP).

v2 design (graph-sharded, pair-compacted, fused skip):
  - Shard by GRAPH: core m owns graphs [16m, 16m+16). All of a graph's dst
    nodes (batch sorted => contiguous) and the edges pointing at them live on
    core m. No collective needed; each core computes 16 final outputs.
  - Per-edge fused matmul (family A): lhsT = xt [34, 128 edges] stationary,
    rhs = w_aug [34, 128] -> [gate-in | v] per edge in PSUM.
  - sigmoid (ACT, batched 8 subtiles), gated message (DVE), PAIR-2 compaction:
    host pairs same-dst edges into lane-aligned A/B halves; one fp16 2x-mode
    DVE add merges them, halving one-hot builds + scatter matmuls.
  - scatter-add via one-hot matmul into per-bucket PSUM; the skip connection
    (x_dst @ Wskip + bconv) is ONE extra accumulate-matmul into the same PSUM.
  - relu + pooling one-hot per 4-bucket window, pooled [128,16] accumulated in
    PSUM; divide by counts; 4-layer MLP per core; host concatenates [1,16]s.
"""
import sys
import types
import numpy as np

NCORES = 8
G = 128
GPC = 16          # graphs per core
H = 64
F = 16
BUCKET = 128
GRP = 4           # pair-slots per group (= 8 raw subtiles)
WIN = 4           # buckets per PSUM drain window
LAST_EXEC_NS = None


def _install_ntff_shim():
    if 'antenv.axon_hooks' in sys.modules:
        return
    try:
        mod = types.ModuleType('antenv.axon_hooks')
        _h = [None]
        mod.set_axon_ntff_profile_hook = lambda h: _h.__setitem__(0, h)
        mod.get_axon_ntff_profile_hook = lambda: _h[0]
        sys.modules['antenv.axon_hooks'] = mod
        import antenv
        antenv.axon_hooks = mod
        from trn_agent_boot.trn_boot import _ntff_profile_via_ctypes
        mod.set_axon_ntff_profile_hook(
            _ntff_profile_via_ctypes('/opt/axon/libaxon_pjrt.so'))
    except Exception:
        pass


def _prep_relation(x_src, x_dst, src, dst, ea, batch_dst, node_start, D_pad):
    """Host marshalling: per-core pair-compacted edge streams.

    Returns per-core xt [34, S*256] (A|B halves per group of GRP pair-slots),
    ld [128, S] fp16, plus the shared schedule (bucket of each pair-slot).
    """
    nbuck = D_pad // BUCKET
    g_of = batch_dst[dst]
    core_of = g_of // GPC
    local = dst - node_start[core_of]
    order = np.lexsort((local, core_of))
    src_s, ea_s, core_s, loc_s = src[order], ea[order, 0], core_of[order], local[order]
    xs = x_src.astype(np.float16)
    xd = x_dst.astype(np.float16)

    # pass 1: per-core per-bucket pair counts
    pair_counts = np.zeros((NCORES, nbuck), np.int64)
    per_core_data = []
    core_bounds = np.searchsorted(core_s, np.arange(NCORES + 1))
    for m in range(NCORES):
        lo, hi = core_bounds[m], core_bounds[m + 1]
        l = loc_s[lo:hi]
        # degree per local dst (sorted) -> pairs per dst
        uniq, cnt = np.unique(l, return_counts=True)
        npair = (cnt + 1) // 2
        bk = uniq // BUCKET
        np.add.at(pair_counts[m], bk, npair)
        per_core_data.append((src_s[lo:hi], ea_s[lo:hi], l, uniq, cnt, npair))

    psub = np.maximum((pair_counts.max(axis=0) + 127) // 128, 0)  # pair-subtiles per bucket
    S = int(psub.sum())
    Sp = ((S + GRP - 1) // GRP) * GRP  # pad to full groups (pads go to last bucket)
    pstart = np.zeros(nbuck + 1, np.int64)
    pstart[1:] = np.cumsum(psub) * 128

    # bucket of each pair-slot (compile-time schedule)
    bucket_of = np.full(Sp, nbuck - 1, np.int64)
    pos = 0
    for b in range(nbuck):
        bucket_of[pos:pos + psub[b]] = b
        pos += int(psub[b])

    per_core = []
    for m in range(NCORES):
        c_src, c_ea, l, uniq, cnt, npair = per_core_data[m]
        E = len(c_src)
        # pair index assignment: edges of each dst sorted contiguously; edge j
        # of dst -> pair j//2, half j%2
        first = np.zeros(len(uniq) + 1, np.int64)
        first[1:] = np.cumsum(cnt)
        j_in_dst = np.arange(E) - np.repeat(first[:-1], cnt)
        half = j_in_dst % 2
        # running pair id per dst, offset by bucket-relative pair base
        pair_in_dst = j_in_dst // 2
        pair_base_dst = np.zeros(len(uniq), np.int64)  # pair index base within bucket
        bk_u = uniq // BUCKET
        for b in range(nbuck):
            sel = bk_u == b
            pb = npair[sel]
            pair_base_dst[sel] = np.concatenate([[0], np.cumsum(pb)[:-1]]) if len(pb) else pb
        pair_id = np.repeat(pair_base_dst, cnt) + pair_in_dst          # within bucket
        # global pair position = pstart[bucket] + pair_id  (in units of pairs)
        gpair = pstart[np.repeat(bk_u, cnt)] + pair_id
        psl = gpair // 128          # pair-slot index
        lane = gpair % 128
        # column index in xt stream: group g = psl//GRP; within group:
        #   A half: (psl%GRP)*128 + lane ; B half: GRP*128 + same
        grp_i = psl // GRP
        colA = grp_i * (2 * GRP * 128) + (psl % GRP) * 128 + lane
        col = colA + half * (GRP * 128)
        ncol = (Sp // GRP) * (2 * GRP * 128)
        xt = np.zeros((34, ncol), np.float16)
        xt[0:16, col] = xs[c_src].T
        xt[16, col] = c_ea.astype(np.float16)
        xt[17, col] = 1.0
        xt[18:34, col] = xd[np.repeat(uniq, cnt) + node_start[m]].T
        ld = np.full(Sp * 128, -1.0, np.float32)
        # lane -> dst slot within bucket, set once per pair (use half==0 entries)
        sel0 = half == 0
        ld[psl[sel0] * 128 + lane[sel0]] = (np.repeat(uniq, cnt)[sel0] % BUCKET).astype(np.float32)
        per_core.append({
            "xt": xt,
            "ld": np.ascontiguousarray(ld.reshape(Sp, 128).T.astype(np.float16)),
        })
    # last scatter pair-slot per bucket (for stop flags); -1 if no pairs
    last_slot = np.full(nbuck, -1, np.int64)
    for s in range(Sp):
        last_slot[bucket_of[s]] = s
    # buckets with zero pair-subtiles keep last_slot -1
    pos = 0
    have = psub > 0
    return {"nbuck": nbuck, "S": Sp, "bucket_of": bucket_of,
            "last_slot": last_slot, "have": have, "per_core": per_core}


def kernel(**inputs):
    _install_ntff_shim()
    import concourse.bass as bass  # noqa: F401
    import concourse.bacc as bacc
    import concourse.mybir as mybir
    import concourse.tile as tile
    from concourse.bass_utils import run_bass_kernel_spmd

    F32 = mybir.dt.float32
    F16 = mybir.dt.float16
    AF = mybir.ActivationFunctionType
    OP = mybir.AluOpType

    ii = {k: np.asarray(v) for k, v in inputs.items()}
    batch_c = ii["batch_c"].astype(np.int64)
    batch_b = ii["batch_b"].astype(np.int64)

    # per-core node ranges (graph-sharded; batch arrays are sorted)
    gcut = np.arange(0, G + 1, GPC)
    cstart = np.searchsorted(batch_c, gcut)   # [9]
    bstart = np.searchsorted(batch_b, gcut)
    Dc_pad = ((int(np.diff(cstart).max()) + BUCKET - 1) // BUCKET) * BUCKET
    Db_pad = ((int(np.diff(bstart).max()) + BUCKET - 1) // BUCKET) * BUCKET

    rel_c = _prep_relation(ii["x_x"], ii["x_c"], ii["src_ac"].astype(np.int64),
                           ii["dst_ac"].astype(np.int64), ii["ea_ac"],
                           batch_c, cstart, Dc_pad)
    rel_b = _prep_relation(ii["x_c"], ii["x_b"], ii["src_cb"].astype(np.int64),
                           ii["dst_cb"].astype(np.int64), ii["ea_cb"],
                           batch_b, bstart, Db_pad)

    def phase_a(x_dst, start, D_pad, m):
        lo, hi = start[m], start[m + 1]
        a = np.zeros((17, D_pad), np.float16)
        a[0:16, :hi - lo] = x_dst[lo:hi].T.astype(np.float16)
        a[16, :hi - lo] = 1.0
        return a

    def batch_layout(batch, start, D_pad, m):
        lo, hi = start[m], start[m + 1]
        nb = D_pad // BUCKET
        padded = np.full(nb * BUCKET, -1.0, np.float32)
        padded[:hi - lo] = batch[lo:hi] - m * GPC   # local graph id 0..15
        return np.ascontiguousarray(padded.reshape(nb, BUCKET).T.astype(np.float16))

    cnt_c = np.bincount(batch_c, minlength=G).astype(np.float32)
    cnt_b = np.bincount(batch_b, minlength=G).astype(np.float32)

    def recip_core(m):
        r = np.zeros((128, GPC), np.float32)
        r[0:64, :] = 1.0 / np.maximum(cnt_c[m * GPC:(m + 1) * GPC], 1.0)
        r[64:128, :] = 1.0 / np.maximum(cnt_b[m * GPC:(m + 1) * GPC], 1.0)
        return r

    def waug(rel):
        Wq, Wv, Wk = ii[f"Wq_{rel}"], ii[f"Wv_{rel}"], ii[f"Wk_{rel}"]
        We = ii[f"We_{rel}"][0]
        bq, bv, bk, be = (ii[f"bq_{rel}"], ii[f"bv_{rel}"],
                          ii[f"bk_{rel}"], ii[f"be_{rel}"])
        w = np.zeros((34, 128), np.float32)
        w[0:16, 0:64] = Wq; w[0:16, 64:128] = Wv
        w[16, 0:64] = 2 * We; w[16, 64:128] = We
        w[17, 0:64] = bq + bk + 2 * be; w[17, 64:128] = bv + be
        w[18:34, 0:64] = Wk
        return w.astype(np.float16)

    def wskip(rel):
        w = np.zeros((17, 64), np.float32)
        w[0:16] = ii[f"Wskip_{rel}"]
        w[16] = ii[f"bconv_{rel}"]
        return w.astype(np.float16)

    iota4 = np.ascontiguousarray(
        np.tile(np.arange(BUCKET, dtype=np.float16), (128, GRP, 1)))
    iotag4 = np.ascontiguousarray(
        np.tile(np.arange(GPC, dtype=np.float16), (128, WIN, 1)))
    mlp_w = {
        "W1": ii["W1"].astype(np.float16), "W2": ii["W2"].astype(np.float16),
        "W3": ii["W3"].astype(np.float16), "Wout": ii["Wout"].astype(np.float16),
        "b1": ii["b1"].astype(np.float32).reshape(64, 1),
        "b2": ii["b2"].astype(np.float32).reshape(64, 1),
        "b3": ii["b3"].astype(np.float32).reshape(64, 1),
        "bout": ii["bout"].astype(np.float32).reshape(1, 1),
    }

    # ---------------- device program ----------------
    nc = bacc.Bacc("TRN2", target_bir_lowering=False, debug=False,
                   num_devices=NCORES)

    def din(name, arr0):
        return nc.dram_tensor(name, list(arr0.shape),
                              mybir.dt.from_np(arr0.dtype), kind="ExternalInput")

    h = {}
    h["xt_c"] = din("xt_c", rel_c["per_core"][0]["xt"])
    h["xt_b"] = din("xt_b", rel_b["per_core"][0]["xt"])
    h["ld_c"] = din("ld_c", rel_c["per_core"][0]["ld"])
    h["ld_b"] = din("ld_b", rel_b["per_core"][0]["ld"])
    h["pa_c"] = din("pa_c", phase_a(ii["x_c"], cstart, Dc_pad, 0))
    h["pa_b"] = din("pa_b", phase_a(ii["x_b"], bstart, Db_pad, 0))
    h["bt_c"] = din("bt_c", batch_layout(batch_c, cstart, Dc_pad, 0))
    h["bt_b"] = din("bt_b", batch_layout(batch_b, bstart, Db_pad, 0))
    h["waug_c"] = din("waug_c", waug("ac"))
    h["waug_b"] = din("waug_b", waug("cb"))
    h["wskip_c"] = din("wskip_c", wskip("ac"))
    h["wskip_b"] = din("wskip_b", wskip("cb"))
    h["iota4"] = din("iota4", iota4)
    h["iotag4"] = din("iotag4", iotag4)
    h["recip"] = din("recip", recip_core(0))
    for k, v in mlp_w.items():
        h["mlp_" + k] = din("mlp_" + k, v)
    out_h = nc.dram_tensor("out", [1, GPC], F32, kind="ExternalOutput")

    with tile.TileContext(nc) as tc:
        with tc.tile_pool(name="const", bufs=1) as cp, \
             tc.tile_pool(name="acc", bufs=1) as accp, \
             tc.tile_pool(name="stream", bufs=3) as sp, \
             tc.tile_pool(name="work", bufs=3) as wp, \
             tc.tile_pool(name="psum", bufs=2, space="PSUM") as pp, \
             tc.tile_pool(name="psB", bufs=2, space="PSUM") as ppB, \
             tc.tile_pool(name="psA", bufs=1, space="PSUM") as ppA:

            iota4_t = cp.tile([128, GRP, BUCKET], F16, tag="iota4_t")
            nc.sync.dma_start(iota4_t[:], h["iota4"].ap())
            iotag4_t = cp.tile([128, WIN, GPC], F16, tag="iotag4_t")
            nc.sync.dma_start(iotag4_t[:], h["iotag4"].ap())

            pooled_ps = ppA.tile([128, GPC], F32, tag="pooled_ps")

            def relation(tag, rel, D_pad, row_off):
                nbuck = rel["nbuck"]
                S = rel["S"]
                bucket_of = rel["bucket_of"]
                last_slot = rel["last_slot"]
                w_t = cp.tile([34, 128], F16, name=f"waug_{tag}", tag=f"waug_{tag}")
                nc.sync.dma_start(w_t[:], h[f"waug_{tag}"].ap())
                ws_t = cp.tile([17, 64], F16, name=f"wskip_{tag}", tag=f"wskip_{tag}")
                nc.sync.dma_start(ws_t[:], h[f"wskip_{tag}"].ap())
                pa_sb = accp.tile([17, D_pad], F16, name=f"pa_{tag}", tag=f"pa_{tag}")
                nc.sync.dma_start(pa_sb[:], h[f"pa_{tag}"].ap())
                ld_sb = accp.tile([128, S], F16, name=f"ld_{tag}", tag=f"ld_{tag}")
                nc.sync.dma_start(ld_sb[:], h[f"ld_{tag}"].ap())
                bt_sb = accp.tile([128, nbuck], F16, name=f"bt_{tag}", tag=f"bt_{tag}")
                nc.sync.dma_start(bt_sb[:], h[f"bt_{tag}"].ap())

                xt_v = h[f"xt_{tag}"].ap()
                nwin = (nbuck + WIN - 1) // WIN
                bps = {}        # window -> psum tile
                opened = set()  # buckets whose psum region is cleared
                drained = set()

                def open_bucket(b):
                    w = b // WIN
                    if w not in bps:
                        bps[w] = ppB.tile([128, WIN, 64], F32,
                                          name=f"bps_{tag}_{w}", tag="bps")
                    # skip connection: one accumulate matmul seeds the region
                    nc.tensor.matmul(bps[w][:, b % WIN, :],
                                     pa_sb[:, b * BUCKET:(b + 1) * BUCKET],
                                     ws_t[:], start=True,
                                     stop=(last_slot[b] < 0),
                                     skip_group_check=True)
                    opened.add(b)

                def drain_window(w):
                    b0 = w * WIN
                    nb = min(WIN, nbuck - b0)
                    for b in range(b0, b0 + nb):
                        if b not in opened:
                            open_bucket(b)
                    h4 = wp.tile([128, WIN, 64], F16, name=f"h4_{tag}_{w}",
                                 tag="h4")
                    nc.vector.tensor_scalar_max(h4[:, :nb, :],
                                                bps[w][:, :nb, :], 0.0)
                    ohg = wp.tile([128, WIN, GPC], F16, name=f"ohg_{tag}_{w}",
                                  tag="ohg")
                    bt3 = bt_sb[:, b0:b0 + nb].rearrange("p (a o) -> p a o", o=1)
                    nc.vector.tensor_tensor(
                        ohg[:, :nb, :], iotag4_t[:, :nb, :],
                        bt3.broadcast_to([128, nb, GPC]), op=OP.is_equal)
                    for i in range(nb):
                        b = b0 + i
                        nc.tensor.matmul(pooled_ps[row_off:row_off + 64, :],
                                         h4[:, i, :], ohg[:, i, :],
                                         start=(b == 0), stop=(b == nbuck - 1),
                                         skip_group_check=True)
                    del bps[w]
                    drained.add(w)

                ngroups = S // GRP
                for g in range(ngroups):
                    xt_t = sp.tile([34, 2 * GRP * 128], F16,
                                   name=f"xt_{tag}_{g}", tag="xt")
                    nc.sync.dma_start(
                        xt_t[:], xt_v[:, g * 2 * GRP * 128:(g + 1) * 2 * GRP * 128])
                    sv = pp.tile([128, 2 * GRP, 128], F32, name=f"sv_{tag}_{g}",
                                 tag="sv")
                    for j in range(2 * GRP):
                        nc.tensor.matmul(sv[:, j, :],
                                         xt_t[:, j * 128:(j + 1) * 128],
                                         w_t[:], start=True, stop=True)
                    gt = wp.tile([128, 2 * GRP, 64], F16, name=f"gt_{tag}_{g}",
                                 tag="gt")
                    nc.scalar.activation(gt[:], sv[:, :, 0:64], AF.Sigmoid)
                    msg = wp.tile([128, 2 * GRP, 64], F16, name=f"msg_{tag}_{g}",
                                  tag="msg")
                    nc.vector.tensor_tensor(msg[:], gt[:], sv[:, :, 64:128],
                                            op=OP.mult)
                    msum = wp.tile([128, GRP, 64], F16, name=f"msum_{tag}_{g}",
                                   tag="msum")
                    nc.vector.tensor_tensor(msum[:], msg[:, 0:GRP, :],
                                            msg[:, GRP:2 * GRP, :], op=OP.add)
                    oh = wp.tile([128, GRP, BUCKET], F16, name=f"oh_{tag}_{g}",
                                 tag="oh")
                    ld3 = ld_sb[:, g * GRP:(g + 1) * GRP].rearrange(
                        "p (a o) -> p a o", o=1)
                    nc.vector.tensor_tensor(oh[:], iota4_t[:],
                                            ld3.broadcast_to([128, GRP, BUCKET]),
                                            op=OP.is_equal)
                    for t in range(GRP):
                        s = g * GRP + t
                        b = int(bucket_of[s])
                        if b not in opened:
                            # drain any completed earlier window first
                            w = b // WIN
                            for wprev in [k for k in list(bps.keys()) if k < w]:
                                drain_window(wprev)
                            open_bucket(b)
                        nc.tensor.matmul(bps[b // WIN][:, b % WIN, :],
                                         oh[:, t, :], msum[:, t, :],
                                         start=False, stop=(last_slot[b] == s),
                                         skip_group_check=True)
                for w in sorted(bps.keys()):
                    drain_window(w)
                # windows whose buckets had no edges at all still need
                # skip + pooling for their nodes:
                for w in range(nwin):
                    if w not in drained:
                        drain_window(w)

            relation("c", rel_c, Dc_pad, 0)
            relation("b", rel_b, Db_pad, 64)

            recip_sb = accp.tile([128, GPC], F32, tag="recip_sb")
            nc.sync.dma_start(recip_sb[:], h["recip"].ap())
            mean_sb = accp.tile([128, GPC], F16, tag="mean_sb")
            nc.vector.tensor_tensor(mean_sb[:], pooled_ps[:], recip_sb[:],
                                    op=OP.mult)

            mw, mb = {}, {}
            for k in ("W1", "W2", "W3", "Wout"):
                mw[k] = accp.tile(list(mlp_w[k].shape), F16, name=f"mw{k}",
                                  tag=f"mw{k}")
                nc.sync.dma_start(mw[k][:], h["mlp_" + k].ap())
            for k in ("b1", "b2", "b3", "bout"):
                mb[k] = accp.tile(list(mlp_w[k].shape), F32, name=f"mb{k}",
                                  tag=f"mb{k}")
                nc.sync.dma_start(mb[k][:], h["mlp_" + k].ap())

            hcur = mean_sb
            for li, (wk, bk) in enumerate((("W1", "b1"), ("W2", "b2"),
                                           ("W3", "b3"))):
                ps = ppA.tile([64, GPC], F32, name=f"mlp{li}", tag="mlps")
                nc.tensor.matmul(ps[:], mw[wk][:], hcur[:], start=True, stop=True)
                hn = accp.tile([64, GPC], F16, name=f"hn{li}", tag=f"hn{li}")
                nc.scalar.activation(hn[:], ps[:], AF.Relu, bias=mb[bk][:])
                hcur = hn
            ps_o = ppA.tile([1, GPC], F32, tag="mlps")
            nc.tensor.matmul(ps_o[:], mw["Wout"][:], hcur[:], start=True, stop=True)
            osb = accp.tile([1, GPC], F32, tag="osb")
            nc.scalar.activation(osb[:], ps_o[:], AF.Identity, bias=mb["bout"][:])
            nc.sync.dma_start(out_h.ap(), osb[:])

    nc.compile()

    in_maps = []
    for m in range(NCORES):
        in_maps.append({
            "xt_c": rel_c["per_core"][m]["xt"],
            "xt_b": rel_b["per_core"][m]["xt"],
            "ld_c": rel_c["per_core"][m]["ld"],
            "ld_b": rel_b["per_core"][m]["ld"],
            "pa_c": phase_a(ii["x_c"], cstart, Dc_pad, m),
            "pa_b": phase_a(ii["x_b"], bstart, Db_pad, m),
            "bt_c": batch_layout(batch_c, cstart, Dc_pad, m),
            "bt_b": batch_layout(batch_b, bstart, Db_pad, m),
            "waug_c": waug("ac"), "waug_b": waug("cb"),
            "wskip_c": wskip("ac"), "wskip_b": wskip("cb"),
            "iota4": iota4, "iotag4": iotag4, "recip": recip_core(m),
            **{"mlp_" + k: v for k, v in mlp_w.items()},
        })
    import os
    trace = bool(os.environ.get("KERNEL_TRACE"))
    res = run_bass_kernel_spmd(nc, in_maps, core_ids=list(range(NCORES)),
                               trace=trace)
    global LAST_EXEC_NS
    LAST_EXEC_NS = res.exec_time_ns
    out = np.concatenate([np.asarray(res.results[m]["out"]).reshape(GPC)
                          for m in range(NCORES)])
    return out.astype(np.float32)
